# revision 34
# baseline (speedup 1.0000x reference)
"""Distributed Trainium2 Bass kernel for 16-head attention.

Reference op: B=2, S=2048, D=1024, H=16 multi-head attention with an
elementwise 0/1 mask, computed as
    out = softmax(mask((q Wq^T)(k Wk^T)^T / sqrt(64))) (v Wv^T) Wo^T

Sharding over 8 NeuronCores: core c handles batch c//4 and head group
c%4 (4 heads = 256 channels). Attention is computed fully locally in a
"dual" layout (scores transposed, [k, q]); the context is exchanged
with one small AllGather per 512-query tile inside each 4-core batch
group, and the output projection is split along the OUTPUT feature dim
(each core holds a 256-column slice of Wo^T), so the host-side unshard
is a pure concatenation.

v2 changes vs the 315us baseline:
  - scores matmuls use PE row tiling (tile_position via base_partition):
    each head's K=64 contraction runs in one half of the array, two
    heads concurrently, instead of zero-padded 128-contraction KTz
    tiles.  Halves the scores streaming time and drops the KTz memsets.
  - slots are emitted in 2-slot batches with all 128-row work (ctx,
    outproj, norm broadcast, deferred projections) grouped before the
    64-row scores section, so the PE tiling-mode switch happens twice
    per batch instead of per-matmul.
  - exchange readback DMAs moved from the sync queue to the gpsimd
    queue: their AllGather-completion waits no longer head-of-line
    block the next tile's mask loads (a measured 7.8us PE stall and
    HAM re-throttle per occurrence).
  - cn (pre-exchange ctx) and y output DMAs moved to the vector queue.
  - a ~20-matmul warmup burst on zeroed SBUF runs while the input DMAs
    land, so the PE's HAM clock gate is warm before the projections.
  - V-projection chunks 13..15 are deferred into early attention
    batches (PE there is ACT-bound with slack).
  - persistent srow tiles (memset once) instead of per-norm memsets.
  - final-tile output projection split into a pair-0 wave (runs while
    the last AllGather is in flight) and a pair-1 remainder.

Compute dtype bf16 (TensorE 1 cyc/row), accumulation f32 in PSUM.
fp8 for any attention tensor was tried and rejected: e4m3's ~3.6% RMS
quantization noise does not average out in random-sign dot products and
blows the 2e-2 rel-err budget (measured 3.7e-2).
"""

import sys

sys.path.insert(0, "/opt/trn_rl_repo")

import numpy as np
import ml_dtypes

BF16 = ml_dtypes.bfloat16

B = 2
S = 2048
DM = 1024
DL = 256  # d-model slice per core (4 heads)
HL = 4  # heads per core
DK = 64
P = 128
QT_N = 4  # query tiles of 512
QTS = 512
KC = 16  # key chunks of 128
MC = 8  # contraction chunks of 128 over d_model
GROUPS = [[0, 1, 2, 3], [4, 5, 6, 7]]

_cached = {}


def _build():
    import concourse.bass as bass
    import concourse.mybir as mybir
    from concourse import bacc
    from concourse.tile import TileContext

    fp32 = mybir.dt.float32
    bf16 = mybir.dt.bfloat16

    nc = bacc.Bacc(num_devices=8)

    # All inputs are pre-folded on the host into the exact SBUF layouts
    # (partition-major), so every load is a single contiguous DMA —
    # the sync engine's ~0.6us per-DMA issue cost was the bottleneck
    # of the input phase, not HBM bandwidth.
    qT = nc.dram_tensor("qT", [P, MC * S], bf16, kind="ExternalInput")
    kT = nc.dram_tensor("kT", [P, MC * S], bf16, kind="ExternalInput")
    vT = nc.dram_tensor("vT", [P, MC * S], bf16, kind="ExternalInput")
    maskT = nc.dram_tensor("maskT", [P, QT_N * KC * QTS], bf16, kind="ExternalInput")
    wq = nc.dram_tensor("wq", [P, MC * DL], bf16, kind="ExternalInput")
    wk = nc.dram_tensor("wk", [P, MC * DL], bf16, kind="ExternalInput")
    wv = nc.dram_tensor("wv", [P, MC * DL], bf16, kind="ExternalInput")
    wo = nc.dram_tensor("wo", [P, MC * DL], bf16, kind="ExternalInput")
    y = nc.dram_tensor("y", [S, DL], fp32, kind="ExternalOutput")

    cc_in = [
        [
            nc.dram_tensor(f"cc_in{t}_{p}", [P, QTS], bf16, kind="Internal")
            for p in range(2)
        ]
        for t in range(QT_N)
    ]
    cc_out = [
        [
            nc.dram_tensor(f"cc_out{t}_{p}", [4 * P, QTS], bf16, kind="Internal")
            for p in range(2)
        ]
        for t in range(QT_N)
    ]

    with TileContext(nc) as tc:
        with (
            tc.tile_pool(name="xT", bufs=16) as xT_pool,
            tc.tile_pool(name="w", bufs=4) as w_pool,
            tc.tile_pool(name="qkt", bufs=2) as qkt_pool,
            tc.tile_pool(name="vext", bufs=16) as vext_pool,
            tc.tile_pool(name="mask", bufs=2) as mask_pool,
            tc.tile_pool(name="attn", bufs=4) as attn_pool,
            tc.tile_pool(name="sm", bufs=3) as sm_pool,
            tc.tile_pool(name="ctxn", bufs=3) as ctxn_pool,
            tc.tile_pool(name="ctxg", bufs=4) as ctxg_pool,
            tc.tile_pool(name="ysb", bufs=2) as y_pool,
            tc.tile_pool(name="ps_big", bufs=2, space="PSUM") as ps_big,
            tc.tile_pool(name="ps_acc", bufs=2, space="PSUM") as ps_acc,
            tc.tile_pool(name="ps_out", bufs=2, space="PSUM") as ps_out,
        ):
            # ---- PE warmup: ~20 dummy matmuls on zeroed SBUF, no input
            # deps, so the HAM clock gate warms while input DMAs land.
            wtile = sm_pool.tile([P, 384], bf16, tag="warm")
            nc.vector.memset(wtile[:], 0.0)
            for wi in range(26):
                wps = ps_out.tile([P, 256], fp32, tag="out", name=f"wm{wi}")
                nc.tensor.matmul(
                    wps[:],
                    wtile[:, 0:P],
                    wtile[:, P : P + 256],
                    start=True,
                    stop=True,
                )

            # ---- upfront input DMAs -----------------------------------------
            def load_x(x_dram):
                # one contiguous 512KB DMA per [128, 2048] m-tile
                tiles = []
                for m in range(MC):
                    t_ = xT_pool.tile([P, S], bf16, tag="xT", name=f"x{m}")
                    nc.sync.dma_start(t_[:], x_dram[:, S * m : S * (m + 1)])
                    tiles.append(t_)
                return tiles

            def load_w(dram, nm):
                # one contiguous 512KB DMA; slice per m-chunk at use sites
                t_ = w_pool.tile([P, MC * DL], bf16, tag="w", name=f"w{nm}")
                nc.sync.dma_start(t_[:], dram[:])
                return t_

            def load_mask(t, lo=0, hi=KC, tile=None):
                # contiguous in the host-folded layout; split into 4-chunk
                # pieces so the 2MB tile spreads across DMA queues
                mt_ = tile
                if mt_ is None:
                    mt_ = mask_pool.tile(
                        [P, KC * QTS], bf16, tag="mask", name=f"mask{t}"
                    )
                base = KC * QTS * t
                kc = lo
                while kc < hi:
                    ke = min(kc + 4, hi)
                    nc.sync.dma_start(
                        mt_[:, QTS * kc : QTS * ke],
                        maskT[:, base + QTS * kc : base + QTS * ke],
                    )
                    kc = ke
                return mt_

            # load order follows the consumption order of the pipeline:
            # q/k (+weights) feed the QT/KT projections that gate the first
            # scores, the head of mask0 feeds the first mask-muls, v feeds
            # the in-stream V projection (batches 0-7), the mask tail and
            # wo come last.
            wq_sb = load_w(wq, "q")
            wk_sb = load_w(wk, "k")
            q_sb = load_x(qT)
            k_sb = load_x(kT)
            mts = {0: load_mask(0, 0, 4)}
            wv_sb = load_w(wv, "v")
            v_sb = load_x(vT)
            load_mask(0, 4, KC, tile=mts[0])
            wo_sb = load_w(wo, "o")

            # ---- Q/K projections: out [256, 2048] as 2 tiles [128, 2048].
            # Row layout per pair tile: partitions 0-63 = head 2*pair's Dk
            # rows, 64-127 = head 2*pair+1's.  This feeds the row-tiled
            # scores matmuls directly (no zero padding needed).
            # Only the two chunks the first scores slots need are emitted
            # up front; the rest are deferred into early batches (PJDEF)
            # so the slot pipeline starts as soon as q/k have landed.
            QT_sb = [
                qkt_pool.tile([P, S], bf16, tag="QT", name=f"QT{dt}")
                for dt in range(2)
            ]
            KT_sb = [
                qkt_pool.tile([P, S], bf16, tag="KT", name=f"KT{dt}")
                for dt in range(2)
            ]

            def proj_chunk(x_sb, w_sb, ot, dt, st):
                ps = ps_big.tile([P, 1024], fp32, tag="big")
                for m in range(MC):
                    for sh in range(2):
                        nc.tensor.matmul(
                            ps[:, QTS * sh : QTS * (sh + 1)],
                            w_sb[:, DL * m + P * dt : DL * m + P * (dt + 1)],
                            x_sb[m][
                                :,
                                1024 * st + QTS * sh : 1024 * st + QTS * (sh + 1),
                            ],
                            start=(m == 0),
                            stop=(m == MC - 1),
                        )
                nc.vector.tensor_copy(ot[:, 1024 * st : 1024 * (st + 1)], ps[:])

            # All QT chunks must be emitted up front: the v-tensor loads
            # reuse the q tiles' ring buffers, so a deferred QT chunk
            # (PE queue, behind v-dependent vproj matmuls) waiting to
            # read q_sb would deadlock against the v DMAs.
            proj_chunk(q_sb, wq_sb, QT_sb[0], 0, 0)
            proj_chunk(q_sb, wq_sb, QT_sb[0], 0, 1)
            proj_chunk(q_sb, wq_sb, QT_sb[1], 1, 0)
            proj_chunk(q_sb, wq_sb, QT_sb[1], 1, 1)
            proj_chunk(k_sb, wk_sb, KT_sb[0], 0, 0)
            # deferred K-projection chunks: batch -> list of (dt, st)
            PJDEF = {
                0: [("k", 0, 1)],
                1: [("k", 1, 0)],
                3: [("k", 1, 1)],
            }

            def emit_pjdef(bi):
                for which, dt, st in PJDEF.get(bi, ()):
                    if which == "q":
                        proj_chunk(q_sb, wq_sb, QT_sb[dt], dt, st)
                    else:
                        proj_chunk(k_sb, wk_sb, KT_sb[dt], dt, st)

            # all-ones lhsT for the denominator broadcast matmul (full 128
            # contraction; the srow rhs is zero except its denominator row)
            ones_lhs = sm_pool.tile([P, P], bf16, tag="ones")
            nc.vector.memset(ones_lhs[:], 1.0)
            # persistent zeroed srow tiles, one per h01 (only row DK is
            # rewritten per use)
            srows = []
            for h01 in range(2):
                sr = sm_pool.tile([P, QTS], bf16, tag="srow", name=f"srow{h01}")
                nc.vector.memset(sr[:], 0.0)
                srows.append(sr)

            # ---- V projection -> V_ext tiles [128, 4*65] ([V_h | 1] blocks)
            # Emitted entirely inside attention batches 0-7 (2 chunks per
            # batch, after that batch's scores/ctx) so the serial prologue
            # is only q/k-dependent and the v DMA hides under early slots.
            # Uses the "out" PSUM ring (bc/op tiles appear there later
            # with prompt releases, so the FIFO stays acyclic).
            vext = [None] * KC

            def vproj_chunk(st):
                ps = ps_out.tile([P, QTS], fp32, tag="out", name=f"vp{st}")
                for m in range(MC):
                    nc.tensor.matmul(
                        ps[:, 0:DL],
                        v_sb[m][:, P * st : P * (st + 1)],
                        wv_sb[:, DL * m : DL * (m + 1)],
                        start=(m == 0),
                        stop=(m == MC - 1),
                    )
                ve = vext_pool.tile(
                    [P, HL * (DK + 1)], bf16, tag="vext", name=f"ve{st}"
                )
                nc.vector.memset(ve[:], 1.0)
                for h in range(HL):
                    nc.vector.tensor_copy(
                        ve[:, 65 * h : 65 * h + DK],
                        ps[:, DK * h : DK * (h + 1)],
                    )
                vext[st] = ve

            # ---- attention + exchange + output projection per query tile ----
            def do_readback(t, pairs=(0, 1)):
                # Emission of these is DELAYED until ~7 slots after the
                # AllGather trigger (see rb_steps), so the completion wait
                # is near-zero and never head-of-line blocks the queue.
                ctxg = []
                for p in pairs:
                    cg = ctxg_pool.tile(
                        [P, 4 * QTS], bf16, tag="ctxg", name=f"cg{t}_{p}"
                    )
                    for i in range(4):
                        nc.sync.dma_start(
                            cg[:, QTS * i : QTS * (i + 1)],
                            cc_out[t][p][P * i : P * (i + 1), :],
                        )
                    ctxg.append(cg)
                return ctxg

            DCS = [0, 2, 4, 6, 1, 3, 5, 7]

            def outproj_steps(t, ctxg):
                # Generator of small out-proj work units (2 matmuls each) to
                # interleave into the attention stream.
                state = {}

                def unit(qs, i0):
                    if qs not in state:
                        state[qs] = ps_out.tile(
                            [P, DL], fp32, tag="out", name=f"op{t}_{qs}"
                        )
                    op = state[qs]
                    for i in (i0, i0 + 1):
                        dc = DCS[i]
                        src = ctxg[dc % 2][
                            :,
                            QTS * (dc // 2) + P * qs : QTS * (dc // 2)
                            + P * (qs + 1),
                        ]
                        nc.tensor.matmul(
                            op[:],
                            src,
                            wo_sb[:, DL * dc : DL * (dc + 1)],
                            start=(i == 0),
                            stop=(i == MC - 1),
                        )
                    if i0 + 2 == MC:
                        ys = y_pool.tile(
                            [P, DL], fp32, tag="ysb", name=f"ys{t}_{qs}"
                        )
                        nc.vector.tensor_copy(ys[:], op[:])
                        r = QTS * t + P * qs
                        nc.sync.dma_start(y[r : r + P, :], ys[:])

                for qs in range(4):
                    for i0 in range(0, MC, 2):
                        yield lambda qs=qs, i0=i0: unit(qs, i0)

            # ---- flat slot pipeline over (qtile, pair, group) ----------------
            # 64 scores slots; ctx accumulation trails by 3-4 slots.  Slots
            # are emitted in 2-slot batches: all 128-row-mode PE work (ctx,
            # norm broadcast, outproj, deferred projections) first, then the
            # 64-row-mode scores, so the PE tiling-mode switches twice per
            # batch instead of per matmul.
            ATD = 8
            at_store = {}
            cp_store = {}
            rolling_cols = ATD * QTS

            def emit_scores(u, grp):
                t, pair = divmod(u, 2)
                if grp == 0:
                    at_store[u] = {
                        h01: attn_pool.tile(
                            [P, rolling_cols], bf16, tag="attn",
                            name=f"at{u}_{h01}",
                        )
                        for h01 in range(2)
                    }
                    if pair == 0 and t + 1 < QT_N:
                        mts[t + 1] = load_mask(t + 1)
                at = at_store[u]
                mt = mts[t]
                sp = {}
                for h01 in range(2):
                    sp[h01] = ps_big.tile(
                        [P, 1024], fp32, tag="big", name=f"sp{u}_{grp}_{h01}"
                    )
                for j in range(2):
                    kc = 2 * grp + j
                    for h01 in range(2):
                        rows = slice(DK * h01, DK * (h01 + 1))
                        # K=64 row-tiled matmul: head h01 runs in array
                        # rows 64*h01..64*h01+63 (tile_position derived
                        # from base_partition), both heads concurrent.
                        nc.tensor.matmul(
                            sp[h01][:, QTS * j : QTS * (j + 1)],
                            KT_sb[pair][rows, P * kc : P * (kc + 1)],
                            QT_sb[pair][rows, QTS * t : QTS * (t + 1)],
                            start=True,
                            stop=True,
                        )
                roff = (2 * grp % ATD) * QTS
                rsl = slice(roff, roff + 1024)
                gsl = slice(1024 * grp, 1024 * (grp + 1))
                for h01 in range(2):
                    nc.scalar.activation(
                        at[h01][:, rsl],
                        sp[h01][:],
                        mybir.ActivationFunctionType.Exp,
                    )
                    nc.vector.tensor_mul(at[h01][:, rsl], at[h01][:, rsl], mt[:, gsl])

            def emit_ctx(u, grp):
                t, pair = divmod(u, 2)
                if grp == 0:
                    cp_store[u] = {
                        h01: ps_acc.tile(
                            [P, QTS], fp32, tag="acc", name=f"cp{u}_{h01}"
                        )
                        for h01 in range(2)
                    }
                at = at_store[u]
                cp = cp_store[u]
                for j in range(2):
                    kc = 2 * grp + j
                    roff = (kc % ATD) * QTS
                    for h01 in range(2):
                        h = 2 * pair + h01
                        nc.tensor.matmul(
                            cp[h01][0 : DK + 1, :],
                            vext[kc][:, 65 * h : 65 * h + DK + 1],
                            at[h01][:, roff : roff + QTS],
                            start=(kc == 0),
                            stop=(kc == KC - 1),
                        )

            def emit_norm(u):
                t, pair = divmod(u, 2)
                cp = cp_store[u]
                for h01 in range(2):
                    # srow is zero except the denominator row, so the all-ones
                    # full-128 matmul broadcasts that row to all partitions
                    srow = srows[h01]
                    nc.vector.tensor_copy(
                        srow[DK : DK + 1, :], cp[h01][DK : DK + 1, :]
                    )
                    bc = ps_out.tile(
                        [P, QTS], fp32, tag="out", name=f"bc{u}_{h01}"
                    )
                    nc.tensor.matmul(
                        bc[:],
                        ones_lhs[:],
                        srow[:],
                        start=True,
                        stop=True,
                    )
                    recipb = sm_pool.tile(
                        [P, QTS], fp32, tag="recipb", bufs=2,
                        name=f"recipb{u}_{h01}",
                    )
                    nc.vector.reciprocal_approx_fast(out=recipb[:], in_=bc[:])
                    cn = ctxn_pool.tile(
                        [DK, QTS], bf16, tag="ctxn", name=f"cn{u}_{h01}"
                    )
                    nc.vector.tensor_mul(
                        cn[:], cp[h01][0:DK, :], recipb[0:DK, :]
                    )
                    nc.gpsimd.dma_start(
                        cc_in[t][pair][DK * h01 : DK * (h01 + 1), :], cn[:]
                    )
                nc.gpsimd.collective_compute(
                    "AllGather",
                    mybir.AluOpType.bypass,
                    replica_groups=GROUPS,
                    ins=[cc_in[t][pair][:]],
                    outs=[cc_out[t][pair][:]],
                )
                del cp_store[u], at_store[u]

            op_steps = []
            rb_steps = []
            NSLOT = 8 * 2 * QT_N
            ctx_done = 0
            cur_slot = [0]

            def emit_ctx_flat(lag):
                ul, gl = divmod(lag, 8)
                emit_ctx(ul, gl)
                if gl == 7:
                    emit_norm(ul)
                    tl, pl = divmod(ul, 2)
                    if pl == 1 and tl < QT_N - 1:
                        # delay readback emission until the AllGather is
                        # (almost) done so its wait never blocks the queue
                        rb_steps.append((cur_slot[0] + 9, tl))

            def pop_rb(i0):
                while rb_steps and rb_steps[0][0] <= i0:
                    _, tl = rb_steps.pop(0)
                    ctxg_t = do_readback(tl)
                    rel = i0 + 9
                    op_steps.extend(
                        (rel, st) for st in outproj_steps(tl, ctxg_t)
                    )

            for bi in range(NSLOT // 2):
                i0 = 2 * bi
                cur_slot[0] = i0
                # --- leftover 128-row-mode work from the previous batch ---
                pop_rb(i0)
                popped = 0
                while op_steps and popped < 3 and op_steps[0][0] <= i0:
                    op_steps.pop(0)[1]()
                    popped += 1
                # --- 64-row-mode scores (keeps the exp pipeline fed) ---
                u, g0 = divmod(i0, 8)
                emit_scores(u, g0)
                emit_scores(u, g0 + 1)
                # --- 128-row-mode: deferred q/k projection chunks, the
                # in-stream V projection (4 chunks per batch over batches
                # 0-3: ctx consumes 4 vext per batch one batch later),
                # then ctx (must reach i0-1 before the next batch's scores
                # reuse the rolling attn window; i0 in the final batches
                # so the last AllGathers trigger early) ---
                emit_pjdef(bi)
                if bi < 4:
                    for st in range(4 * bi, 4 * bi + 4):
                        vproj_chunk(st)
                target = i0 - 1 if bi < NSLOT // 2 - 2 else i0
                while ctx_done <= target and ctx_done < NSLOT:
                    emit_ctx_flat(ctx_done)
                    ctx_done += 1

            # flush remaining ctx / norms (triggers the last AllGathers)
            while ctx_done < NSLOT:
                emit_ctx_flat(ctx_done)
                ctx_done += 1
            # drain pending readbacks and leftover outproj steps (t=2)
            pop_rb(10**9)
            for _, st_ in op_steps:
                st_()
            op_steps = []

            # ---- final tile: pair-0 outproj wave runs while AllGather(3,1)
            # is still in flight; pair-1 remainder after its readback.
            ctxg3 = [None, None]
            ctxg3[0] = do_readback(QT_N - 1, pairs=(0,))[0]
            ctxg3[1] = do_readback(QT_N - 1, pairs=(1,))[0]

            state3 = {}

            def unit3(qs, i0):
                if qs not in state3:
                    state3[qs] = ps_out.tile(
                        [P, DL], fp32, tag="out", name=f"op3_{qs}"
                    )
                op = state3[qs]
                for i in (i0, i0 + 1):
                    dc = DCS[i]
                    src = ctxg3[dc % 2][
                        :,
                        QTS * (dc // 2) + P * qs : QTS * (dc // 2) + P * (qs + 1),
                    ]
                    nc.tensor.matmul(
                        op[:],
                        src,
                        wo_sb[:, DL * dc : DL * (dc + 1)],
                        start=(i == 0),
                        stop=(i == MC - 1),
                    )
                if i0 + 2 == MC:
                    ys = y_pool.tile(
                        [P, DL], fp32, tag="ysb", name=f"ys3_{qs}"
                    )
                    nc.vector.tensor_copy(ys[:], op[:])
                    r = QTS * (QT_N - 1) + P * qs
                    nc.sync.dma_start(y[r : r + P, :], ys[:])

            # wave A: pair-0 chunks for qs 0,1 (only needs ctxg3[0])
            for qs in (0, 1):
                unit3(qs, 0)
                unit3(qs, 2)
            # wave B: complete qs 0,1 then qs 2,3 fully
            for qs in (0, 1):
                unit3(qs, 4)
                unit3(qs, 6)
            for qs in (2, 3):
                for i0 in range(0, MC, 2):
                    unit3(qs, i0)

    nc.compile()
    return nc


def _get_nc():
    if "nc" not in _cached:
        _cached["nc"] = _build()
    return _cached["nc"]


def _fold_x(xT):
    # [DM, S] -> [128, MC*S] with F[p, m*S+c] = xT[128m+p, c]
    return np.ascontiguousarray(
        xT.reshape(MC, P, S).transpose(1, 0, 2).reshape(P, MC * S)
    )


def _fold_w(wT):
    # [DM, DL] -> [128, MC*DL] with F[p, m*DL+c] = wT[128m+p, c]
    return np.ascontiguousarray(
        wT.reshape(MC, P, DL).transpose(1, 0, 2).reshape(P, MC * DL)
    )


def _fold_mask(mT):
    # [S, S] ([k, q]) -> [128, QT_N*KC*QTS] with
    # F[p, t*KC*QTS + kc*QTS + c] = mT[128kc+p, 512t+c]
    return np.ascontiguousarray(
        mT.reshape(KC, P, QT_N, QTS).transpose(1, 2, 0, 3).reshape(P, -1)
    )


def _shard_inputs(q, k, v, mask, w_q, w_k, w_v, w_o):
    in_maps = []
    scale = 1.0 / np.sqrt(DK)
    wqT = (w_q.astype(np.float64) * scale).astype(np.float32).T  # [DM, DM]
    wkT = w_k.T
    wvT = w_v.T
    woT = w_o.T
    for c in range(8):
        b, g = c // 4, c % 4
        sl = slice(DL * g, DL * (g + 1))
        in_maps.append(
            {
                "qT": _fold_x(q[b].T.astype(BF16)),
                "kT": _fold_x(k[b].T.astype(BF16)),
                "vT": _fold_x(v[b].T.astype(BF16)),
                "maskT": _fold_mask(mask[b].T.astype(BF16)),
                "wq": _fold_w(wqT[:, sl].astype(BF16)),
                "wk": _fold_w(wkT[:, sl].astype(BF16)),
                "wv": _fold_w(wvT[:, sl].astype(BF16)),
                "wo": _fold_w(woT[:, sl].astype(BF16)),
            }
        )
    return in_maps


def kernel(q, k, v, mask, w_q, w_k, w_v, w_o, _trace=False, _tmpdir=None):
    from concourse import bass_utils

    nc = _get_nc()
    in_maps = _shard_inputs(q, k, v, mask, w_q, w_k, w_v, w_o)
    res = bass_utils.run_bass_kernel_spmd(
        nc,
        in_maps,
        core_ids=list(range(8)),
        trace=_trace,
        tmpdir=_tmpdir,
    )
    out = np.empty((B, S, DM), dtype=np.float32)
    for c in range(8):
        b, g = c // 4, c % 4
        out[b, :, DL * g : DL * (g + 1)] = res.results[c]["y"]
    if _trace:
        _cached["last_exec_time_ns"] = res.exec_time_ns
        _cached["last_results"] = res
    return out
